# revision 2
# baseline (speedup 1.0000x reference)
"""Trainium2 Bass kernel for nn_ExpertLayer (MoE top-2 routing).

Strategy (expert-parallel, routed):
- Host: gate logits/softmax/top-2 in numpy, dispatch tokens to experts.
  Each of the 8 NeuronCores owns one expert's W1/b1/W2/b2 and receives the
  tokens routed to it (padded to a multiple of 512).
- Device (per core): YT = W2.T @ gelu(W1.T @ XT + b1) + b2 in feature-major
  layout ([feature, token]), fp32r matmuls (full-rate PE, ~1.5e-4 rel err),
  512-token tiles, weights streamed from HBM with double buffering.
- Host: scatter-add scaled expert outputs (combine weights) into the output.

The reference computes all 8 experts densely for every token; top-2 combine
weights zero out the rest, so the routed computation is exactly equivalent
(up to fp rounding) at 1/4 the FLOPs.
"""

import numpy as np

import concourse.bass as bass
import concourse.mybir as mybir
from concourse.bass import ts
from concourse.bass_utils import run_bass_kernel_spmd
from concourse.tile import TileContext

D_MODEL = 1024
D_FF = 4096
N_EXPERTS = 8
TOP_K = 2
GATE_TEMP = 1.0
LOAD_BALANCE_WEIGHT = 0.01

TILE_N = 512          # tokens per device tile (one PSUM bank at fp32)
DC = D_MODEL // 128   # 8 d_model chunks
FC = D_FF // 128      # 32 d_ff chunks
MC = D_MODEL // 128   # 8 output chunks

TRACE = False         # set by test harness for NTFF profiling
LAST_RESULT = None    # BassKernelResults of the last run (for test harness)


def _spread_waits(nc: bass.Bass) -> None:
    """Walrus codegen rejects instructions carrying more than one sync wait.
    Move excess waits onto same-engine NoOp carriers inserted immediately
    before the offending instruction (same-engine program order preserves
    the wait-before-execute semantics)."""
    for func in nc.m.functions:
        for bb in func.blocks:
            il = bb.instructions
            i = 0
            while i < len(il):
                inst = il[i]
                si = getattr(inst, "sync_info", None)
                if si is not None:
                    waits = list(si.on_wait)
                    if len(waits) > 1:
                        for w in waits[:-1]:
                            nop = mybir.InstNoOp(
                                name=nc.get_next_instruction_name()
                            )
                            nop.engine = inst.engine
                            nop.sync_info = mybir.SyncInfo(
                                on_wait=[w], on_update=[]
                            )
                            il.insert(i, nop)
                            i += 1
                        inst.sync_info = mybir.SyncInfo(
                            on_wait=[waits[-1]], on_update=list(si.on_update)
                        )
                i += 1


def _build_nc(nt: int) -> bass.Bass:
    """FFN for one expert over nt*TILE_N tokens, feature-major layout."""
    f32 = mybir.dt.float32
    f32r = mybir.dt.float32r
    C = nt * TILE_N

    nc = bass.Bass()
    xt = nc.declare_dram_parameter("xt", [D_MODEL, C], f32r, isOutput=False)
    w1 = nc.declare_dram_parameter("w1", [D_MODEL, D_FF], f32r, isOutput=False)
    b1t = nc.declare_dram_parameter("b1t", [128, FC], f32, isOutput=False)
    w2 = nc.declare_dram_parameter("w2", [D_FF, D_MODEL], f32r, isOutput=False)
    b2t = nc.declare_dram_parameter("b2t", [128, MC], f32, isOutput=False)
    yt = nc.declare_dram_parameter("yt", [D_MODEL, C], f32, isOutput=True)

    xt_r = xt.rearrange("(dc p) n -> p dc n", p=128)
    w1_r = w1.rearrange("(dc p) f -> p dc f", p=128)
    w2_r = w2.rearrange("(fc p) m -> p fc m", p=128)
    yt_r = yt.rearrange("(mc p) n -> p mc n", p=128)

    Act = mybir.ActivationFunctionType

    with TileContext(nc) as tc:
        with (
            tc.tile_pool(name="consts", bufs=1) as consts,
            tc.tile_pool(name="xpool", bufs=2) as xpool,
            tc.tile_pool(name="w1pool", bufs=3) as w1pool,
            tc.tile_pool(name="w2pool", bufs=2) as w2pool,
            tc.tile_pool(name="htpool", bufs=1) as htpool,
            tc.tile_pool(name="ypool", bufs=3) as ypool,
            tc.tile_pool(name="psh", bufs=4, space="PSUM") as psh,
            tc.tile_pool(name="psy", bufs=3, space="PSUM") as psy,
        ):
            b1_sb = consts.tile([128, FC], f32)
            nc.sync.dma_start(out=b1_sb, in_=b1t[:, :])
            b2_sb = consts.tile([128, MC], f32)
            nc.sync.dma_start(out=b2_sb, in_=b2t[:, :])

            for t in range(nt):
                xt_sb = xpool.tile([128, DC, TILE_N], f32r, tag="xt_sb")
                nc.sync.dma_start(out=xt_sb, in_=xt_r[:, :, ts(t, TILE_N)])

                ht = htpool.tile([128, FC, TILE_N], f32r, tag="ht")
                for f in range(FC):
                    w1f = w1pool.tile([128, DC, 128], f32r, tag="w1f")
                    nc.sync.dma_start(out=w1f, in_=w1_r[:, :, ts(f, 128)])
                    ph = psh.tile([128, TILE_N], f32, tag="ph")
                    for dc in range(DC):
                        nc.tensor.matmul(
                            ph,
                            w1f[:, dc, :],
                            xt_sb[:, dc, :],
                            start=(dc == 0),
                            stop=(dc == DC - 1),
                        )
                    nc.scalar.activation(
                        ht[:, f, :], ph, Act.Gelu, bias=b1_sb[:, f : f + 1]
                    )

                for m in range(MC):
                    w2m = w2pool.tile([128, FC, 128], f32r, tag="w2m")
                    nc.sync.dma_start(out=w2m, in_=w2_r[:, :, ts(m, 128)])
                    py = psy.tile([128, TILE_N], f32, tag="py")
                    for fc in range(FC):
                        nc.tensor.matmul(
                            py,
                            w2m[:, fc, :],
                            ht[:, fc, :],
                            start=(fc == 0),
                            stop=(fc == FC - 1),
                        )
                    y_sb = ypool.tile([128, TILE_N], f32, tag="y_sb")
                    nc.scalar.activation(
                        y_sb, py, Act.Identity, bias=b2_sb[:, m : m + 1]
                    )
                    nc.sync.dma_start(out=yt_r[:, m, ts(t, TILE_N)], in_=y_sb)

    _spread_waits(nc)
    return nc


def kernel(x, gate_w, W1, b1, W2, b2):
    global LAST_RESULT
    x = np.asarray(x, dtype=np.float32)
    gate_w = np.asarray(gate_w, dtype=np.float32)
    W1 = np.asarray(W1, dtype=np.float32)
    b1 = np.asarray(b1, dtype=np.float32)
    W2 = np.asarray(W2, dtype=np.float32)
    b2 = np.asarray(b2, dtype=np.float32)

    B, S, D = x.shape
    N = B * S
    E = gate_w.shape[0]
    xf = x.reshape(N, D)

    # ---- Gate: softmax + top-2 (host, fp32, matches jax semantics) ----
    logits = (xf @ gate_w.T) / np.float32(GATE_TEMP)
    m = logits.max(axis=1, keepdims=True)
    ex = np.exp(logits - m, dtype=np.float32)
    probs = ex / ex.sum(axis=1, keepdims=True, dtype=np.float32)
    order = np.argsort(-probs, axis=1, kind="stable")[:, :TOP_K]
    topk = np.take_along_axis(probs, order, axis=1)
    topk = topk / topk.sum(axis=1, keepdims=True, dtype=np.float32)

    # ---- Aux loss (faithful to reference) ----
    prob_mean = probs.mean(axis=0, dtype=np.float32)
    tokens_per_expert = (probs > 0).astype(np.float32).mean(axis=0, dtype=np.float32)
    aux_loss = np.float32(
        np.sum(prob_mean * tokens_per_expert, dtype=np.float32)
        * np.float32(E)
        * np.float32(LOAD_BALANCE_WEIGHT)
    )

    # ---- Dispatch: token lists per expert ----
    tok_lists = []
    wgt_lists = []
    for e in range(E):
        sel_k0 = order[:, 0] == e
        sel_k1 = order[:, 1] == e
        toks = np.nonzero(sel_k0 | sel_k1)[0]
        w = np.where(sel_k0[toks], topk[toks, 0], topk[toks, 1])
        tok_lists.append(toks)
        wgt_lists.append(w.astype(np.float32))
    cmax = max(max(len(t) for t in tok_lists), 1)
    nt = (cmax + TILE_N - 1) // TILE_N
    C = nt * TILE_N

    in_maps = []
    for e in range(E):
        toks = tok_lists[e]
        xt_e = np.zeros((D_MODEL, C), dtype=np.float32)
        xt_e[:, : len(toks)] = xf[toks].T
        in_maps.append(
            {
                "xt": np.ascontiguousarray(xt_e),
                "w1": np.ascontiguousarray(W1[e]),
                "b1t": np.ascontiguousarray(b1[e].reshape(FC, 128).T),
                "w2": np.ascontiguousarray(W2[e]),
                "b2t": np.ascontiguousarray(b2[e].reshape(MC, 128).T),
            }
        )

    nc = _build_nc(nt)
    if TRACE:
        res = run_bass_kernel_spmd(
            nc, in_maps, core_ids=list(range(N_EXPERTS)), trace=True
        )
    else:
        res = run_bass_kernel_spmd(nc, in_maps, core_ids=list(range(N_EXPERTS)))
    LAST_RESULT = res

    out = np.zeros((N, D_MODEL), dtype=np.float32)
    for e in range(E):
        toks = tok_lists[e]
        if len(toks) == 0:
            continue
        y_e = res.results[e]["yt"][:, : len(toks)].T
        out[toks] += wgt_lists[e][:, None] * y_e
    return out.reshape(B, S, D_MODEL), aux_loss


# revision 4
# speedup vs baseline: 1.0665x; 1.0665x over previous
"""Trainium2 Bass kernel for nn_ExpertLayer (MoE top-2 routing).

Strategy (expert-parallel, routed):
- Host: gate logits/softmax/top-2 in numpy, dispatch tokens to experts.
  Each of the 8 NeuronCores owns one expert's W1/b1/W2/b2 and receives the
  tokens routed to it (padded to a multiple of 512).
- Device (per core): YT = W2.T @ gelu(W1.T @ XT + b1) + b2 in feature-major
  layout ([feature, token]), fp32r matmuls (full-rate PE, ~1.5e-4 rel err),
  512-token tiles, weights streamed from HBM with double buffering.
- Host: scatter-add scaled expert outputs (combine weights) into the output.

The reference computes all 8 experts densely for every token; top-2 combine
weights zero out the rest, so the routed computation is exactly equivalent
(up to fp rounding) at 1/4 the FLOPs.
"""

import numpy as np

import concourse.bass as bass
import concourse.mybir as mybir
from concourse.bass import ts
from concourse.bass_utils import run_bass_kernel_spmd
from concourse.tile import TileContext

D_MODEL = 1024
D_FF = 4096
N_EXPERTS = 8
TOP_K = 2
GATE_TEMP = 1.0
LOAD_BALANCE_WEIGHT = 0.01

TILE_N = 512          # tokens per device tile (one PSUM bank at fp32)
DC = D_MODEL // 128   # 8 d_model chunks
FC = D_FF // 128      # 32 d_ff chunks
MC = D_MODEL // 128   # 8 output chunks

TRACE = False         # set by test harness for NTFF profiling
LAST_RESULT = None    # BassKernelResults of the last run (for test harness)


def _spread_waits(nc: bass.Bass) -> None:
    """Walrus codegen rejects instructions carrying more than one sync wait.
    Move excess waits onto same-engine NoOp carriers inserted immediately
    before the offending instruction (same-engine program order preserves
    the wait-before-execute semantics)."""
    for func in nc.m.functions:
        for bb in func.blocks:
            il = bb.instructions
            i = 0
            while i < len(il):
                inst = il[i]
                si = getattr(inst, "sync_info", None)
                if si is not None:
                    waits = list(si.on_wait)
                    if len(waits) > 1:
                        for w in waits[:-1]:
                            nop = mybir.InstNoOp(
                                name=nc.get_next_instruction_name()
                            )
                            nop.engine = inst.engine
                            nop.sync_info = mybir.SyncInfo(
                                on_wait=[w], on_update=[]
                            )
                            il.insert(i, nop)
                            i += 1
                        inst.sync_info = mybir.SyncInfo(
                            on_wait=[waits[-1]], on_update=list(si.on_update)
                        )
                i += 1


def _build_nc(nt: int) -> bass.Bass:
    """FFN for one expert over nt*TILE_N tokens, feature-major layout."""
    f32 = mybir.dt.float32
    f32r = mybir.dt.float32r
    C = nt * TILE_N

    nc = bass.Bass()
    xt = nc.declare_dram_parameter("xt", [D_MODEL, C], f32r, isOutput=False)
    w1 = nc.declare_dram_parameter("w1", [D_MODEL, D_FF], f32r, isOutput=False)
    b1t = nc.declare_dram_parameter("b1t", [128, FC], f32, isOutput=False)
    w2 = nc.declare_dram_parameter("w2", [D_FF, D_MODEL], f32r, isOutput=False)
    b2t = nc.declare_dram_parameter("b2t", [128, MC], f32, isOutput=False)
    yt = nc.declare_dram_parameter("yt", [D_MODEL, C], f32, isOutput=True)

    xt_r = xt.rearrange("(dc p) n -> p dc n", p=128)
    w1_r = w1.rearrange("(dc p) f -> p dc f", p=128)
    w2_r = w2.rearrange("(fc p) m -> p fc m", p=128)
    yt_r = yt.rearrange("(mc p) n -> p mc n", p=128)

    Act = mybir.ActivationFunctionType

    with TileContext(nc) as tc:
        with (
            tc.tile_pool(name="consts", bufs=1) as consts,
            tc.tile_pool(name="xpool", bufs=2) as xpool,
            tc.tile_pool(name="w1pool", bufs=2) as w1pool,
            tc.tile_pool(name="w2pool", bufs=2) as w2pool,
            tc.tile_pool(name="htpool", bufs=1) as htpool,
            tc.tile_pool(name="ypool", bufs=3) as ypool,
            tc.tile_pool(name="psh", bufs=4, space="PSUM") as psh,
            tc.tile_pool(name="psy", bufs=3, space="PSUM") as psy,
        ):
            b1_sb = consts.tile([128, FC], f32)
            nc.sync.dma_start(out=b1_sb, in_=b1t[:, :])
            b2_sb = consts.tile([128, MC], f32)
            nc.sync.dma_start(out=b2_sb, in_=b2t[:, :])

            for t in range(nt):
                xt_sb = xpool.tile([128, DC, TILE_N], f32r, tag="xt_sb")
                nc.sync.dma_start(out=xt_sb, in_=xt_r[:, :, ts(t, TILE_N)])

                ht = htpool.tile([128, FC, TILE_N], f32r, tag="ht")
                for fp in range(FC // 2):
                    # double-width weight loads: 1KB contiguous runs per
                    # partition instead of 512B → better DMA packet size
                    w1f = w1pool.tile([128, DC, 256], f32r, tag="w1f")
                    nc.sync.dma_start(out=w1f, in_=w1_r[:, :, ts(fp, 256)])
                    for fi in range(2):
                        f = 2 * fp + fi
                        ph = psh.tile([128, TILE_N], f32, tag="ph")
                        for dc in range(DC):
                            nc.tensor.matmul(
                                ph,
                                w1f[:, dc, ts(fi, 128)],
                                xt_sb[:, dc, :],
                                start=(dc == 0),
                                stop=(dc == DC - 1),
                            )
                        nc.scalar.activation(
                            ht[:, f, :], ph, Act.Gelu, bias=b1_sb[:, f : f + 1]
                        )

                for mp in range(MC // 2):
                    w2m = w2pool.tile([128, FC, 256], f32r, tag="w2m")
                    nc.sync.dma_start(out=w2m, in_=w2_r[:, :, ts(mp, 256)])
                    for mi in range(2):
                        m = 2 * mp + mi
                        py = psy.tile([128, TILE_N], f32, tag="py")
                        for fc in range(FC):
                            nc.tensor.matmul(
                                py,
                                w2m[:, fc, ts(mi, 128)],
                                ht[:, fc, :],
                                start=(fc == 0),
                                stop=(fc == FC - 1),
                            )
                        y_sb = ypool.tile([128, TILE_N], f32, tag="y_sb")
                        nc.scalar.activation(
                            y_sb, py, Act.Identity, bias=b2_sb[:, m : m + 1]
                        )
                        nc.sync.dma_start(
                            out=yt_r[:, m, ts(t, TILE_N)], in_=y_sb
                        )

    _spread_waits(nc)
    return nc


def kernel(x, gate_w, W1, b1, W2, b2):
    global LAST_RESULT
    x = np.asarray(x, dtype=np.float32)
    gate_w = np.asarray(gate_w, dtype=np.float32)
    W1 = np.asarray(W1, dtype=np.float32)
    b1 = np.asarray(b1, dtype=np.float32)
    W2 = np.asarray(W2, dtype=np.float32)
    b2 = np.asarray(b2, dtype=np.float32)

    B, S, D = x.shape
    N = B * S
    E = gate_w.shape[0]
    xf = x.reshape(N, D)

    # ---- Gate: softmax + top-2 (host, fp32, matches jax semantics) ----
    logits = (xf @ gate_w.T) / np.float32(GATE_TEMP)
    m = logits.max(axis=1, keepdims=True)
    ex = np.exp(logits - m, dtype=np.float32)
    probs = ex / ex.sum(axis=1, keepdims=True, dtype=np.float32)
    order = np.argsort(-probs, axis=1, kind="stable")[:, :TOP_K]
    topk = np.take_along_axis(probs, order, axis=1)
    topk = topk / topk.sum(axis=1, keepdims=True, dtype=np.float32)

    # ---- Aux loss (faithful to reference) ----
    prob_mean = probs.mean(axis=0, dtype=np.float64)
    tokens_per_expert = (probs > 0).mean(axis=0, dtype=np.float64)
    aux_loss = np.float32(
        np.sum(prob_mean * tokens_per_expert) * E * LOAD_BALANCE_WEIGHT
    )

    # ---- Dispatch: token lists per expert ----
    tok_lists = []
    wgt_lists = []
    for e in range(E):
        sel_k0 = order[:, 0] == e
        sel_k1 = order[:, 1] == e
        toks = np.nonzero(sel_k0 | sel_k1)[0]
        w = np.where(sel_k0[toks], topk[toks, 0], topk[toks, 1])
        tok_lists.append(toks)
        wgt_lists.append(w.astype(np.float32))
    cmax = max(max(len(t) for t in tok_lists), 1)
    nt = (cmax + TILE_N - 1) // TILE_N
    C = nt * TILE_N

    in_maps = []
    for e in range(E):
        toks = tok_lists[e]
        xt_e = np.zeros((D_MODEL, C), dtype=np.float32)
        xt_e[:, : len(toks)] = xf[toks].T
        in_maps.append(
            {
                "xt": np.ascontiguousarray(xt_e),
                "w1": np.ascontiguousarray(W1[e]),
                "b1t": np.ascontiguousarray(b1[e].reshape(FC, 128).T),
                "w2": np.ascontiguousarray(W2[e]),
                "b2t": np.ascontiguousarray(b2[e].reshape(MC, 128).T),
            }
        )

    nc = _build_nc(nt)
    if TRACE:
        res = run_bass_kernel_spmd(
            nc, in_maps, core_ids=list(range(N_EXPERTS)), trace=True
        )
    else:
        res = run_bass_kernel_spmd(nc, in_maps, core_ids=list(range(N_EXPERTS)))
    LAST_RESULT = res

    out = np.zeros((N, D_MODEL), dtype=np.float32)
    for e in range(E):
        toks = tok_lists[e]
        if len(toks) == 0:
            continue
        y_e = res.results[e]["yt"][:, : len(toks)].T
        out[toks] += wgt_lists[e][:, None] * y_e
    return out.reshape(B, S, D_MODEL), aux_loss


# revision 5
# speedup vs baseline: 1.2115x; 1.1359x over previous
"""Trainium2 Bass kernel for nn_ExpertLayer (MoE top-2 routing).

Strategy (expert-parallel, routed):
- Host: gate logits/softmax/top-2 in numpy, dispatch tokens to experts.
  Each of the 8 NeuronCores owns one expert's W1/b1/W2/b2 and receives the
  tokens routed to it (padded to a multiple of 512).
- Device (per core): YT = W2.T @ gelu(W1.T @ XT + b1) + b2 in feature-major
  layout ([feature, token]), fp32r matmuls (full-rate PE, ~1.5e-4 rel err),
  512-token tiles, weights streamed from HBM with double buffering.
- Host: scatter-add scaled expert outputs (combine weights) into the output.

The reference computes all 8 experts densely for every token; top-2 combine
weights zero out the rest, so the routed computation is exactly equivalent
(up to fp rounding) at 1/4 the FLOPs.
"""

import numpy as np

import concourse.bass as bass
import concourse.mybir as mybir
from concourse.bass import ts
from concourse.bass_utils import run_bass_kernel_spmd
from concourse.tile import TileContext

D_MODEL = 1024
D_FF = 4096
N_EXPERTS = 8
TOP_K = 2
GATE_TEMP = 1.0
LOAD_BALANCE_WEIGHT = 0.01

TILE_N = 512          # tokens per device tile (one PSUM bank at fp32)
DC = D_MODEL // 128   # 8 d_model chunks
FC = D_FF // 128      # 32 d_ff chunks
MC = D_MODEL // 128   # 8 output chunks

TRACE = False         # set by test harness for NTFF profiling
LAST_RESULT = None    # BassKernelResults of the last run (for test harness)


def _spread_waits(nc: bass.Bass) -> None:
    """Walrus codegen rejects instructions carrying more than one sync wait.
    Move excess waits onto same-engine NoOp carriers inserted immediately
    before the offending instruction (same-engine program order preserves
    the wait-before-execute semantics)."""
    for func in nc.m.functions:
        for bb in func.blocks:
            il = bb.instructions
            i = 0
            while i < len(il):
                inst = il[i]
                si = getattr(inst, "sync_info", None)
                if si is not None:
                    waits = list(si.on_wait)
                    if len(waits) > 1:
                        for w in waits[:-1]:
                            nop = mybir.InstNoOp(
                                name=nc.get_next_instruction_name()
                            )
                            nop.engine = inst.engine
                            nop.sync_info = mybir.SyncInfo(
                                on_wait=[w], on_update=[]
                            )
                            il.insert(i, nop)
                            i += 1
                        inst.sync_info = mybir.SyncInfo(
                            on_wait=[waits[-1]], on_update=list(si.on_update)
                        )
                i += 1


def _build_nc(nt: int) -> bass.Bass:
    """FFN for one expert over nt*TILE_N tokens, feature-major layout."""
    f32 = mybir.dt.float32
    f32r = mybir.dt.float32r
    C = nt * TILE_N

    nc = bass.Bass()
    xt = nc.declare_dram_parameter("xt", [D_MODEL, C], f32r, isOutput=False)
    w1 = nc.declare_dram_parameter("w1", [D_MODEL, D_FF], f32r, isOutput=False)
    b1t = nc.declare_dram_parameter("b1t", [128, FC], f32, isOutput=False)
    w2 = nc.declare_dram_parameter("w2", [D_FF, D_MODEL], f32r, isOutput=False)
    b2t = nc.declare_dram_parameter("b2t", [128, MC], f32, isOutput=False)
    yt = nc.declare_dram_parameter("yt", [D_MODEL, C], f32, isOutput=True)

    xt_r = xt.rearrange("(dc p) n -> p dc n", p=128)
    w1_r = w1.rearrange("(dc p) f -> p dc f", p=128)
    w2_r = w2.rearrange("(fc p) m -> p fc m", p=128)
    yt_r = yt.rearrange("(mc p) n -> p mc n", p=128)

    Act = mybir.ActivationFunctionType

    with TileContext(nc) as tc:
        with (
            tc.tile_pool(name="consts", bufs=1) as consts,
            tc.tile_pool(name="xpool", bufs=2) as xpool,
            tc.tile_pool(name="w1pool", bufs=4) as w1pool,
            tc.tile_pool(name="w2pool", bufs=2) as w2pool,
            tc.tile_pool(name="htpool", bufs=1) as htpool,
            tc.tile_pool(name="ypool", bufs=3) as ypool,
            tc.tile_pool(name="psh", bufs=4, space="PSUM") as psh,
            tc.tile_pool(name="psy", bufs=3, space="PSUM") as psy,
        ):
            b1_sb = consts.tile([128, FC], f32)
            nc.sync.dma_start(out=b1_sb, in_=b1t[:, :])
            b2_sb = consts.tile([128, MC], f32)
            nc.sync.dma_start(out=b2_sb, in_=b2t[:, :])

            for t in range(nt):
                xt_sb = xpool.tile([128, DC, TILE_N], f32r, tag="xt_sb")
                nc.sync.dma_start(out=xt_sb, in_=xt_r[:, :, ts(t, TILE_N)])

                ht = htpool.tile([128, FC, TILE_N], f32r, tag="ht")
                for fp in range(FC // 2):
                    # double-width weight loads: 1KB contiguous runs per
                    # partition instead of 512B → better DMA packet size
                    w1f = w1pool.tile([128, DC, 256], f32r, tag="w1f")
                    nc.sync.dma_start(out=w1f, in_=w1_r[:, :, ts(fp, 256)])
                    for fi in range(2):
                        f = 2 * fp + fi
                        ph = psh.tile([128, TILE_N], f32, tag="ph")
                        for dc in range(DC):
                            nc.tensor.matmul(
                                ph,
                                w1f[:, dc, ts(fi, 128)],
                                xt_sb[:, dc, :],
                                start=(dc == 0),
                                stop=(dc == DC - 1),
                            )
                        nc.scalar.activation(
                            ht[:, f, :], ph, Act.Gelu, bias=b1_sb[:, f : f + 1]
                        )

                for mp in range(MC // 2):
                    w2a = w2pool.tile([128, FC // 2, 256], f32r, tag="w2a")
                    nc.sync.dma_start(out=w2a, in_=w2_r[:, : FC // 2, ts(mp, 256)])
                    w2b = w2pool.tile([128, FC // 2, 256], f32r, tag="w2b")
                    nc.sync.dma_start(out=w2b, in_=w2_r[:, FC // 2 :, ts(mp, 256)])
                    for mi in range(2):
                        m = 2 * mp + mi
                        py = psy.tile([128, TILE_N], f32, tag="py")
                        for fc in range(FC):
                            w2sel = w2a if fc < FC // 2 else w2b
                            nc.tensor.matmul(
                                py,
                                w2sel[:, fc % (FC // 2), ts(mi, 128)],
                                ht[:, fc, :],
                                start=(fc == 0),
                                stop=(fc == FC - 1),
                            )
                        y_sb = ypool.tile([128, TILE_N], f32, tag="y_sb")
                        nc.scalar.activation(
                            y_sb, py, Act.Identity, bias=b2_sb[:, m : m + 1]
                        )
                        nc.sync.dma_start(
                            out=yt_r[:, m, ts(t, TILE_N)], in_=y_sb
                        )

    _spread_waits(nc)
    return nc


def kernel(x, gate_w, W1, b1, W2, b2):
    global LAST_RESULT
    x = np.asarray(x, dtype=np.float32)
    gate_w = np.asarray(gate_w, dtype=np.float32)
    W1 = np.asarray(W1, dtype=np.float32)
    b1 = np.asarray(b1, dtype=np.float32)
    W2 = np.asarray(W2, dtype=np.float32)
    b2 = np.asarray(b2, dtype=np.float32)

    B, S, D = x.shape
    N = B * S
    E = gate_w.shape[0]
    xf = x.reshape(N, D)

    # ---- Gate: softmax + top-2 (host, fp32, matches jax semantics) ----
    logits = (xf @ gate_w.T) / np.float32(GATE_TEMP)
    m = logits.max(axis=1, keepdims=True)
    ex = np.exp(logits - m, dtype=np.float32)
    probs = ex / ex.sum(axis=1, keepdims=True, dtype=np.float32)
    order = np.argsort(-probs, axis=1, kind="stable")[:, :TOP_K]
    topk = np.take_along_axis(probs, order, axis=1)
    topk = topk / topk.sum(axis=1, keepdims=True, dtype=np.float32)

    # ---- Aux loss (faithful to reference) ----
    prob_mean = probs.mean(axis=0, dtype=np.float64)
    tokens_per_expert = (probs > 0).mean(axis=0, dtype=np.float64)
    aux_loss = np.float32(
        np.sum(prob_mean * tokens_per_expert) * E * LOAD_BALANCE_WEIGHT
    )

    # ---- Dispatch: token lists per expert ----
    tok_lists = []
    wgt_lists = []
    for e in range(E):
        sel_k0 = order[:, 0] == e
        sel_k1 = order[:, 1] == e
        toks = np.nonzero(sel_k0 | sel_k1)[0]
        w = np.where(sel_k0[toks], topk[toks, 0], topk[toks, 1])
        tok_lists.append(toks)
        wgt_lists.append(w.astype(np.float32))
    cmax = max(max(len(t) for t in tok_lists), 1)
    nt = (cmax + TILE_N - 1) // TILE_N
    C = nt * TILE_N

    in_maps = []
    for e in range(E):
        toks = tok_lists[e]
        xt_e = np.zeros((D_MODEL, C), dtype=np.float32)
        xt_e[:, : len(toks)] = xf[toks].T
        in_maps.append(
            {
                "xt": np.ascontiguousarray(xt_e),
                "w1": np.ascontiguousarray(W1[e]),
                "b1t": np.ascontiguousarray(b1[e].reshape(FC, 128).T),
                "w2": np.ascontiguousarray(W2[e]),
                "b2t": np.ascontiguousarray(b2[e].reshape(MC, 128).T),
            }
        )

    nc = _build_nc(nt)
    if TRACE:
        res = run_bass_kernel_spmd(
            nc, in_maps, core_ids=list(range(N_EXPERTS)), trace=True
        )
    else:
        res = run_bass_kernel_spmd(nc, in_maps, core_ids=list(range(N_EXPERTS)))
    LAST_RESULT = res

    out = np.zeros((N, D_MODEL), dtype=np.float32)
    for e in range(E):
        toks = tok_lists[e]
        if len(toks) == 0:
            continue
        y_e = res.results[e]["yt"][:, : len(toks)].T
        out[toks] += wgt_lists[e][:, None] * y_e
    return out.reshape(B, S, D_MODEL), aux_loss


# revision 7
# speedup vs baseline: 1.2215x; 1.0083x over previous
"""Trainium2 Bass kernel for nn_ExpertLayer (MoE top-2 routing).

Strategy (expert-parallel, routed):
- Host: gate logits/softmax/top-2 in numpy, dispatch tokens to experts.
  Each of the 8 NeuronCores owns one expert's W1/b1/W2/b2 and receives the
  tokens routed to it (padded to a multiple of 512).
- Device (per core): YT = W2.T @ gelu(W1.T @ XT + b1) + b2 in feature-major
  layout ([feature, token]), fp32r matmuls (full-rate PE, ~1.5e-4 rel err),
  512-token tiles, weights streamed from HBM with double buffering.
- Host: scatter-add scaled expert outputs (combine weights) into the output.

The reference computes all 8 experts densely for every token; top-2 combine
weights zero out the rest, so the routed computation is exactly equivalent
(up to fp rounding) at 1/4 the FLOPs.
"""

import numpy as np

import concourse.bass as bass
import concourse.mybir as mybir
from concourse.bass import ts
from concourse.bass_utils import run_bass_kernel_spmd
from concourse.tile import TileContext

D_MODEL = 1024
D_FF = 4096
N_EXPERTS = 8
TOP_K = 2
GATE_TEMP = 1.0
LOAD_BALANCE_WEIGHT = 0.01

TILE_N = 512          # tokens per device tile (one PSUM bank at fp32)
DC = D_MODEL // 128   # 8 d_model chunks
FC = D_FF // 128      # 32 d_ff chunks
MC = D_MODEL // 128   # 8 output chunks

TRACE = False         # set by test harness for NTFF profiling
LAST_RESULT = None    # BassKernelResults of the last run (for test harness)


def _spread_waits(nc: bass.Bass) -> None:
    """Walrus codegen rejects instructions carrying more than one sync wait.
    Move excess waits onto same-engine NoOp carriers inserted immediately
    before the offending instruction (same-engine program order preserves
    the wait-before-execute semantics)."""
    for func in nc.m.functions:
        for bb in func.blocks:
            il = bb.instructions
            i = 0
            while i < len(il):
                inst = il[i]
                si = getattr(inst, "sync_info", None)
                if si is not None:
                    waits = list(si.on_wait)
                    if len(waits) > 1:
                        for w in waits[:-1]:
                            nop = mybir.InstNoOp(
                                name=nc.get_next_instruction_name()
                            )
                            nop.engine = inst.engine
                            nop.sync_info = mybir.SyncInfo(
                                on_wait=[w], on_update=[]
                            )
                            il.insert(i, nop)
                            i += 1
                        inst.sync_info = mybir.SyncInfo(
                            on_wait=[waits[-1]], on_update=list(si.on_update)
                        )
                i += 1


def _build_nc(nt: int) -> bass.Bass:
    """FFN for one expert over nt*TILE_N tokens, feature-major layout."""
    f32 = mybir.dt.float32
    f32r = mybir.dt.float32r
    C = nt * TILE_N

    nc = bass.Bass()
    xt = nc.declare_dram_parameter("xt", [D_MODEL, C], f32r, isOutput=False)
    w1 = nc.declare_dram_parameter("w1", [D_MODEL, D_FF], f32r, isOutput=False)
    b1t = nc.declare_dram_parameter("b1t", [128, FC], f32, isOutput=False)
    w2 = nc.declare_dram_parameter("w2", [D_FF, D_MODEL], f32r, isOutput=False)
    b2t = nc.declare_dram_parameter("b2t", [128, MC], f32, isOutput=False)
    yt = nc.declare_dram_parameter("yt", [D_MODEL, C], f32, isOutput=True)

    xt_r = xt.rearrange("(dc p) n -> p dc n", p=128)
    w1_r = w1.rearrange("(dc p) f -> p dc f", p=128)
    w2_r = w2.rearrange("(fc p) m -> p fc m", p=128)
    yt_r = yt.rearrange("(mc p) n -> p mc n", p=128)

    Act = mybir.ActivationFunctionType

    with TileContext(nc) as tc:
        with (
            tc.tile_pool(name="consts", bufs=1) as consts,
            tc.tile_pool(name="xpool", bufs=2) as xpool,
            tc.tile_pool(name="w1pool", bufs=4) as w1pool,
            tc.tile_pool(name="w2pool", bufs=2) as w2pool,
            tc.tile_pool(name="htpool", bufs=1) as htpool,
            tc.tile_pool(name="ypool", bufs=3) as ypool,
            tc.tile_pool(name="psh", bufs=4, space="PSUM") as psh,
            tc.tile_pool(name="psy", bufs=3, space="PSUM") as psy,
        ):
            b1_sb = consts.tile([128, FC], f32)
            nc.sync.dma_start(out=b1_sb, in_=b1t[:, :])
            b2_sb = consts.tile([128, MC], f32)
            nc.sync.dma_start(out=b2_sb, in_=b2t[:, :])

            # software-pipelined loads: w1 pair DMAs issued 2 groups ahead of
            # use (crossing tile boundaries), xt issued one tile ahead, so
            # MM1 never starts a tile with cold buffers.
            from collections import deque

            NPAIR = FC // 2
            w1_queue = deque()

            def issue_w1(fp):
                w1f = w1pool.tile([128, DC, 256], f32r, tag="w1f")
                nc.sync.dma_start(out=w1f, in_=w1_r[:, :, ts(fp, 256)])
                w1_queue.append(w1f)

            xt_tiles = {}

            def issue_xt(t):
                tile = xpool.tile([128, DC, TILE_N], f32r, tag="xt_sb")
                nc.sync.dma_start(out=tile, in_=xt_r[:, :, ts(t, TILE_N)])
                xt_tiles[t] = tile

            issue_xt(0)
            issue_w1(0)
            issue_w1(1)

            for t in range(nt):
                xt_sb = xt_tiles.pop(t)

                ht = htpool.tile([128, FC, TILE_N], f32r, tag="ht")
                for fp in range(FC // 2):
                    pos = t * NPAIR + fp + 2
                    if pos < nt * NPAIR:
                        issue_w1(pos % NPAIR)
                    w1f = w1_queue.popleft()
                    for fi in range(2):
                        f = 2 * fp + fi
                        ph = psh.tile([128, TILE_N], f32, tag="ph")
                        for dc in range(DC):
                            nc.tensor.matmul(
                                ph,
                                w1f[:, dc, ts(fi, 128)],
                                xt_sb[:, dc, :],
                                start=(dc == 0),
                                stop=(dc == DC - 1),
                            )
                        nc.scalar.activation(
                            ht[:, f, :], ph, Act.Gelu, bias=b1_sb[:, f : f + 1]
                        )

                if t + 1 < nt:
                    issue_xt(t + 1)

                for mp in range(MC // 2):
                    w2a = w2pool.tile([128, FC // 2, 256], f32r, tag="w2a")
                    nc.sync.dma_start(out=w2a, in_=w2_r[:, : FC // 2, ts(mp, 256)])
                    w2b = w2pool.tile([128, FC // 2, 256], f32r, tag="w2b")
                    nc.sync.dma_start(out=w2b, in_=w2_r[:, FC // 2 :, ts(mp, 256)])
                    for mi in range(2):
                        m = 2 * mp + mi
                        py = psy.tile([128, TILE_N], f32, tag="py")
                        for fc in range(FC):
                            w2sel = w2a if fc < FC // 2 else w2b
                            nc.tensor.matmul(
                                py,
                                w2sel[:, fc % (FC // 2), ts(mi, 128)],
                                ht[:, fc, :],
                                start=(fc == 0),
                                stop=(fc == FC - 1),
                            )
                        y_sb = ypool.tile([128, TILE_N], f32, tag="y_sb")
                        nc.scalar.activation(
                            y_sb, py, Act.Identity, bias=b2_sb[:, m : m + 1]
                        )
                        nc.sync.dma_start(
                            out=yt_r[:, m, ts(t, TILE_N)], in_=y_sb
                        )

    _spread_waits(nc)
    return nc


def kernel(x, gate_w, W1, b1, W2, b2):
    global LAST_RESULT
    x = np.asarray(x, dtype=np.float32)
    gate_w = np.asarray(gate_w, dtype=np.float32)
    W1 = np.asarray(W1, dtype=np.float32)
    b1 = np.asarray(b1, dtype=np.float32)
    W2 = np.asarray(W2, dtype=np.float32)
    b2 = np.asarray(b2, dtype=np.float32)

    B, S, D = x.shape
    N = B * S
    E = gate_w.shape[0]
    xf = x.reshape(N, D)

    # ---- Gate: softmax + top-2 (host, fp32, matches jax semantics) ----
    logits = (xf @ gate_w.T) / np.float32(GATE_TEMP)
    m = logits.max(axis=1, keepdims=True)
    ex = np.exp(logits - m, dtype=np.float32)
    probs = ex / ex.sum(axis=1, keepdims=True, dtype=np.float32)
    order = np.argsort(-probs, axis=1, kind="stable")[:, :TOP_K]
    topk = np.take_along_axis(probs, order, axis=1)
    topk = topk / topk.sum(axis=1, keepdims=True, dtype=np.float32)

    # ---- Aux loss (faithful to reference) ----
    prob_mean = probs.mean(axis=0, dtype=np.float64)
    tokens_per_expert = (probs > 0).mean(axis=0, dtype=np.float64)
    aux_loss = np.float32(
        np.sum(prob_mean * tokens_per_expert) * E * LOAD_BALANCE_WEIGHT
    )

    # ---- Dispatch: token lists per expert ----
    tok_lists = []
    wgt_lists = []
    for e in range(E):
        sel_k0 = order[:, 0] == e
        sel_k1 = order[:, 1] == e
        toks = np.nonzero(sel_k0 | sel_k1)[0]
        w = np.where(sel_k0[toks], topk[toks, 0], topk[toks, 1])
        tok_lists.append(toks)
        wgt_lists.append(w.astype(np.float32))
    cmax = max(max(len(t) for t in tok_lists), 1)
    nt = (cmax + TILE_N - 1) // TILE_N
    C = nt * TILE_N

    in_maps = []
    for e in range(E):
        toks = tok_lists[e]
        xt_e = np.zeros((D_MODEL, C), dtype=np.float32)
        xt_e[:, : len(toks)] = xf[toks].T
        in_maps.append(
            {
                "xt": np.ascontiguousarray(xt_e),
                "w1": np.ascontiguousarray(W1[e]),
                "b1t": np.ascontiguousarray(b1[e].reshape(FC, 128).T),
                "w2": np.ascontiguousarray(W2[e]),
                "b2t": np.ascontiguousarray(b2[e].reshape(MC, 128).T),
            }
        )

    nc = _build_nc(nt)
    if TRACE:
        res = run_bass_kernel_spmd(
            nc, in_maps, core_ids=list(range(N_EXPERTS)), trace=True
        )
    else:
        res = run_bass_kernel_spmd(nc, in_maps, core_ids=list(range(N_EXPERTS)))
    LAST_RESULT = res

    out = np.zeros((N, D_MODEL), dtype=np.float32)
    for e in range(E):
        toks = tok_lists[e]
        if len(toks) == 0:
            continue
        y_e = res.results[e]["yt"][:, : len(toks)].T
        out[toks] += wgt_lists[e][:, None] * y_e
    return out.reshape(B, S, D_MODEL), aux_loss


# revision 28
# speedup vs baseline: 1.3004x; 1.0646x over previous
"""Trainium2 Bass kernel for nn_ExpertLayer (MoE top-2 routing).

Strategy (expert-parallel, routed):
- Host: gate logits/softmax/top-2 in numpy, dispatch tokens to experts.
  Each of the 8 NeuronCores owns one expert's W1/b1/W2/b2 and receives the
  tokens routed to it (padded to a multiple of 512).
- Device (per core): YT = W2.T @ gelu(W1.T @ XT + b1) + b2 in feature-major
  layout ([feature, token]), fp32r matmuls (full-rate PE, ~1.5e-4 rel err),
  512-token tiles, weights streamed from HBM with double buffering.
- Host: scatter-add scaled expert outputs (combine weights) into the output.

The reference computes all 8 experts densely for every token; top-2 combine
weights zero out the rest, so the routed computation is exactly equivalent
(up to fp rounding) at 1/4 the FLOPs.
"""

import numpy as np

import concourse.bass as bass
import concourse.mybir as mybir
from concourse.bass import ts
from concourse.bass_utils import run_bass_kernel_spmd
from concourse.tile import TileContext

D_MODEL = 1024
D_FF = 4096
N_EXPERTS = 8
TOP_K = 2
GATE_TEMP = 1.0
LOAD_BALANCE_WEIGHT = 0.01

TILE_N_MAX = 512      # tokens per device tile (one PSUM bank at fp32)
DC = D_MODEL // 128   # 8 d_model chunks
FC = D_FF // 128      # 32 d_ff chunks
MC = D_MODEL // 128   # 8 output chunks

TRACE = False         # set by test harness for NTFF profiling
LAST_RESULT = None    # BassKernelResults of the last run (for test harness)


def _spread_waits(nc: bass.Bass) -> None:
    """Walrus codegen rejects instructions carrying more than one sync wait.
    Move excess waits onto same-engine NoOp carriers inserted immediately
    before the offending instruction (same-engine program order preserves
    the wait-before-execute semantics)."""
    for func in nc.m.functions:
        for bb in func.blocks:
            il = bb.instructions
            i = 0
            while i < len(il):
                inst = il[i]
                si = getattr(inst, "sync_info", None)
                if si is not None:
                    waits = list(si.on_wait)
                    if len(waits) > 1:
                        for w in waits[:-1]:
                            nop = mybir.InstNoOp(
                                name=nc.get_next_instruction_name()
                            )
                            nop.engine = inst.engine
                            nop.sync_info = mybir.SyncInfo(
                                on_wait=[w], on_update=[]
                            )
                            il.insert(i, nop)
                            i += 1
                        inst.sync_info = mybir.SyncInfo(
                            on_wait=[waits[-1]], on_update=list(si.on_update)
                        )
                i += 1


def _build_nc(nt: int, tile_n: int) -> bass.Bass:
    """FFN for one expert over nt*tile_n tokens, feature-major layout."""
    f32 = mybir.dt.float32
    f32r = mybir.dt.float32r
    TILE_N = tile_n
    C = nt * TILE_N
    # wider w1 loads (2KB runs) fit in SBUF only at smaller tile_n
    W1W = 4 if TILE_N <= 456 else 2
    W1_BUFS = 3 if W1W == 4 else 4

    nc = bass.Bass()
    xt = nc.declare_dram_parameter("xt", [D_MODEL, C], f32r, isOutput=False)
    w1 = nc.declare_dram_parameter("w1", [D_MODEL, D_FF], f32r, isOutput=False)
    b1t = nc.declare_dram_parameter("b1t", [128, FC], f32, isOutput=False)
    w2 = nc.declare_dram_parameter("w2", [D_FF, D_MODEL], f32r, isOutput=False)
    b2t = nc.declare_dram_parameter("b2t", [128, MC], f32, isOutput=False)
    yt = nc.declare_dram_parameter("yt", [D_MODEL, C], f32, isOutput=True)

    xt_r = xt.rearrange("(dc p) n -> p dc n", p=128)
    w1_r = w1.rearrange("(dc p) f -> p dc f", p=128)
    w2_r = w2.rearrange("(fc p) m -> p fc m", p=128)
    yt_r = yt.rearrange("(mc p) n -> p mc n", p=128)

    Act = mybir.ActivationFunctionType

    with TileContext(nc) as tc:
        with (
            tc.tile_pool(name="consts", bufs=1) as consts,
            tc.tile_pool(name="xpool", bufs=2) as xpool,
            tc.tile_pool(name="w1pool", bufs=W1_BUFS) as w1pool,
            tc.tile_pool(name="w2pool", bufs=5) as w2pool,
            tc.tile_pool(name="htpool", bufs=1) as htpool,
            tc.tile_pool(name="ypool", bufs=2) as ypool,
            tc.tile_pool(name="psh", bufs=4, space="PSUM") as psh,
            tc.tile_pool(name="psy", bufs=3, space="PSUM") as psy,
        ):
            b1_sb = consts.tile([128, FC], f32)
            b2_sb = consts.tile([128, MC], f32)

            # software-pipelined loads: w1 pair DMAs issued 2 groups ahead of
            # use (crossing tile boundaries), xt issued one tile ahead, so
            # MM1 never starts a tile with cold buffers.
            from collections import deque

            NPAIR = FC // W1W
            w1_queue = deque()

            def issue_w1(fp, split=False):
                w1f = w1pool.tile([128, DC, 128 * W1W], f32r, tag="w1f")
                if split:
                    # four queues, first 128-col chunk first → the first
                    # matmul group can start as soon as ~0.5MB has landed
                    h = 32 * W1W
                    for i in range(4):
                        nc.sync.dma_start(
                            out=w1f[:, :, i * h : (i + 1) * h],
                            in_=w1_r[
                                :,
                                :,
                                fp * 128 * W1W + i * h : fp * 128 * W1W
                                + (i + 1) * h,
                            ],
                        )
                else:
                    nc.sync.dma_start(out=w1f, in_=w1_r[:, :, ts(fp, 128 * W1W)])
                w1_queue.append(w1f)

            xt_tiles = {}

            def issue_xt(t, split=False):
                tile = xpool.tile([128, DC, TILE_N], f32r, tag="xt_sb")
                nsplit = 4 if split else 2
                step = DC // nsplit
                for i in range(nsplit):
                    nc.sync.dma_start(
                        out=tile[:, i * step : (i + 1) * step, :],
                        in_=xt_r[:, i * step : (i + 1) * step, ts(t, TILE_N)],
                    )
                xt_tiles[t] = tile

            # w2 streaming plan: 3 quarters of the mp=0 group stay RESIDENT
            # across tiles (saves (nt-1)*3MB of HBM re-streaming); the rest
            # stream through a lookahead deque.
            N_RES = 4
            w2_queue = deque()
            w2_issued = [0]
            stream_plan = [
                (tt, mp, q)
                for tt in range(nt)
                for mp in range(MC // 2)
                for q in range(4)
                if not (mp == 0 and q < N_RES)
            ]
            # plan entries consumed strictly in order; index of first entry
            # belonging to each (t, mp) group (group may be empty when all
            # of its quarters are resident):
            group_start = {}
            k = 0
            for tt in range(nt):
                for mp in range(MC // 2):
                    group_start[(tt, mp)] = k
                    k += (4 - N_RES) if mp == 0 else 4

            def issue_w2():
                tt, mp, q = stream_plan[w2_issued[0]]
                w2q = w2pool.tile([128, 8, 256], f32r, tag="w2q")
                nc.sync.dma_start(
                    out=w2q, in_=w2_r[:, q * 8 : (q + 1) * 8, ts(mp, 256)]
                )
                w2_queue.append(w2q)
                w2_issued[0] += 1

            issue_xt(0, split=True)
            issue_w1(0, split=True)
            issue_w1(1)
            nc.scalar.dma_start(out=b1_sb, in_=b1t[:, :])
            nc.scalar.dma_start(out=b2_sb, in_=b2t[:, :])
            w2res = []

            def issue_w2res():
                q = len(w2res)
                w2rq = consts.tile([128, 8, 256], f32r, name=f"w2res{q}")
                nc.sync.dma_start(
                    out=w2rq, in_=w2_r[:, q * 8 : (q + 1) * 8, ts(0, 256)]
                )
                w2res.append(w2rq)

            issue_w2()
            issue_w2()

            for t in range(nt):
                xt_sb = xt_tiles.pop(t)

                ht = htpool.tile([128, FC, TILE_N], f32r, tag="ht")
                tile_q_base = t * (MC // 2) * 4
                for fp in range(NPAIR):
                    pos = t * NPAIR + fp + 2
                    if pos < nt * NPAIR:
                        issue_w1(pos % NPAIR)
                    if t == 0 and NPAIR - 4 <= fp < NPAIR - 4 + N_RES:
                        issue_w2res()
                    w1f = w1_queue.popleft()
                    for fi in range(W1W):
                        f = W1W * fp + fi
                        ph = psh.tile([128, TILE_N], f32, tag="ph")
                        for dc in range(DC):
                            nc.tensor.matmul(
                                ph,
                                w1f[:, dc, ts(fi, 128)],
                                xt_sb[:, dc, :],
                                start=(dc == 0),
                                stop=(dc == DC - 1),
                            )
                        nc.scalar.activation(
                            ht[:, f, :], ph, Act.Gelu, bias=b1_sb[:, f : f + 1]
                        )

                if t + 1 < nt:
                    issue_xt(t + 1)

                for mp in range(MC // 2):
                    nstream = 4 - N_RES if mp == 0 else 4
                    gs = group_start[(t, mp)]
                    while w2_issued[0] < min(len(stream_plan), gs + nstream + 2):
                        issue_w2()
                    if mp == 0:
                        quarters = list(w2res) + [
                            w2_queue.popleft() for _ in range(nstream)
                        ]
                    else:
                        quarters = [w2_queue.popleft() for _ in range(4)]
                    for mi in range(2):
                        m = 2 * mp + mi
                        py = psy.tile([128, TILE_N], f32, tag="py")
                        for fc in range(FC):
                            nc.tensor.matmul(
                                py,
                                quarters[fc // 8][:, fc % 8, ts(mi, 128)],
                                ht[:, fc, :],
                                start=(fc == 0),
                                stop=(fc == FC - 1),
                            )
                        y_sb = ypool.tile([128, TILE_N], f32, tag="y_sb")
                        nc.scalar.activation(
                            y_sb, py, Act.Identity, bias=b2_sb[:, m : m + 1]
                        )
                        nc.scalar.dma_start(
                            out=yt_r[:, m, ts(t, TILE_N)], in_=y_sb
                        )

    _spread_waits(nc)
    return nc


def kernel(x, gate_w, W1, b1, W2, b2):
    global LAST_RESULT
    x = np.asarray(x, dtype=np.float32)
    gate_w = np.asarray(gate_w, dtype=np.float32)
    W1 = np.asarray(W1, dtype=np.float32)
    b1 = np.asarray(b1, dtype=np.float32)
    W2 = np.asarray(W2, dtype=np.float32)
    b2 = np.asarray(b2, dtype=np.float32)

    B, S, D = x.shape
    N = B * S
    E = gate_w.shape[0]
    xf = x.reshape(N, D)

    # ---- Gate: softmax + top-2 (host, fp32, matches jax semantics) ----
    logits = (xf @ gate_w.T) / np.float32(GATE_TEMP)
    m = logits.max(axis=1, keepdims=True)
    ex = np.exp(logits - m, dtype=np.float32)
    probs = ex / ex.sum(axis=1, keepdims=True, dtype=np.float32)
    order = np.argsort(-probs, axis=1, kind="stable")[:, :TOP_K]
    topk = np.take_along_axis(probs, order, axis=1)
    topk = topk / topk.sum(axis=1, keepdims=True, dtype=np.float32)

    # ---- Aux loss (faithful to reference) ----
    prob_mean = probs.mean(axis=0, dtype=np.float64)
    tokens_per_expert = (probs > 0).mean(axis=0, dtype=np.float64)
    aux_loss = np.float32(
        np.sum(prob_mean * tokens_per_expert) * E * LOAD_BALANCE_WEIGHT
    )

    # ---- Dispatch: token lists per expert ----
    tok_lists = []
    wgt_lists = []
    for e in range(E):
        sel_k0 = order[:, 0] == e
        sel_k1 = order[:, 1] == e
        toks = np.nonzero(sel_k0 | sel_k1)[0]
        w = np.where(sel_k0[toks], topk[toks, 0], topk[toks, 1])
        tok_lists.append(toks)
        wgt_lists.append(w.astype(np.float32))
    cmax = max(max(len(t) for t in tok_lists), 256)
    nt = (cmax + TILE_N_MAX - 1) // TILE_N_MAX
    tile_n = max(256, ((cmax + nt - 1) // nt + 7) // 8 * 8)
    C = nt * tile_n

    in_maps = []
    for e in range(E):
        toks = tok_lists[e]
        xt_e = np.zeros((D_MODEL, C), dtype=np.float32)
        xt_e[:, : len(toks)] = xf[toks].T
        in_maps.append(
            {
                "xt": np.ascontiguousarray(xt_e),
                "w1": np.ascontiguousarray(W1[e]),
                "b1t": np.ascontiguousarray(b1[e].reshape(FC, 128).T),
                "w2": np.ascontiguousarray(W2[e]),
                "b2t": np.ascontiguousarray(b2[e].reshape(MC, 128).T),
            }
        )

    nc = _build_nc(nt, tile_n)
    if TRACE:
        res = run_bass_kernel_spmd(
            nc, in_maps, core_ids=list(range(N_EXPERTS)), trace=True
        )
    else:
        res = run_bass_kernel_spmd(nc, in_maps, core_ids=list(range(N_EXPERTS)))
    LAST_RESULT = res

    out = np.zeros((N, D_MODEL), dtype=np.float32)
    for e in range(E):
        toks = tok_lists[e]
        if len(toks) == 0:
            continue
        y_e = res.results[e]["yt"][:, : len(toks)].T
        out[toks] += wgt_lists[e][:, None] * y_e
    return out.reshape(B, S, D_MODEL), aux_loss


# revision 30
# speedup vs baseline: 1.3414x; 1.0315x over previous
"""Trainium2 Bass kernel for nn_ExpertLayer (MoE top-2 routing).

Strategy (expert-parallel, routed):
- Host: gate logits/softmax/top-2 in numpy, dispatch tokens to experts.
  Each of the 8 NeuronCores owns one expert's W1/b1/W2/b2 and receives the
  tokens routed to it (padded to a multiple of 512).
- Device (per core): YT = W2.T @ gelu(W1.T @ XT + b1) + b2 in feature-major
  layout ([feature, token]), fp32r matmuls (full-rate PE, ~1.5e-4 rel err),
  512-token tiles, weights streamed from HBM with double buffering.
- Host: scatter-add scaled expert outputs (combine weights) into the output.

The reference computes all 8 experts densely for every token; top-2 combine
weights zero out the rest, so the routed computation is exactly equivalent
(up to fp rounding) at 1/4 the FLOPs.
"""

import numpy as np

import concourse.bass as bass
import concourse.mybir as mybir
from concourse.bass import ts
from concourse.bass_utils import run_bass_kernel_spmd
from concourse.tile import TileContext

D_MODEL = 1024
D_FF = 4096
N_EXPERTS = 8
TOP_K = 2
GATE_TEMP = 1.0
LOAD_BALANCE_WEIGHT = 0.01

TILE_N_MAX = 512      # tokens per device tile (one PSUM bank at fp32)
DC = D_MODEL // 128   # 8 d_model chunks
FC = D_FF // 128      # 32 d_ff chunks
MC = D_MODEL // 128   # 8 output chunks

TRACE = False         # set by test harness for NTFF profiling
LAST_RESULT = None    # BassKernelResults of the last run (for test harness)


def _spread_waits(nc: bass.Bass) -> None:
    """Walrus codegen rejects instructions carrying more than one sync wait.
    Move excess waits onto same-engine NoOp carriers inserted immediately
    before the offending instruction (same-engine program order preserves
    the wait-before-execute semantics)."""
    for func in nc.m.functions:
        for bb in func.blocks:
            il = bb.instructions
            i = 0
            while i < len(il):
                inst = il[i]
                si = getattr(inst, "sync_info", None)
                if si is not None:
                    waits = list(si.on_wait)
                    if len(waits) > 1:
                        for w in waits[:-1]:
                            nop = mybir.InstNoOp(
                                name=nc.get_next_instruction_name()
                            )
                            nop.engine = inst.engine
                            nop.sync_info = mybir.SyncInfo(
                                on_wait=[w], on_update=[]
                            )
                            il.insert(i, nop)
                            i += 1
                        inst.sync_info = mybir.SyncInfo(
                            on_wait=[waits[-1]], on_update=list(si.on_update)
                        )
                i += 1


def _build_nc(nt: int, tile_n: int) -> bass.Bass:
    """FFN for one expert over nt*tile_n tokens, feature-major layout."""
    f32 = mybir.dt.float32
    f32r = mybir.dt.float32r
    TILE_N = tile_n
    C = nt * TILE_N
    # wider w1 loads (2KB runs) fit in SBUF only at smaller tile_n
    W1W = 4 if TILE_N <= 456 else 2
    W1_BUFS = 3 if W1W == 4 else 4

    nc = bass.Bass()
    xt = nc.declare_dram_parameter("xt", [D_MODEL, C], f32r, isOutput=False)
    w1 = nc.declare_dram_parameter("w1", [D_MODEL, D_FF], f32r, isOutput=False)
    b1t = nc.declare_dram_parameter("b1t", [128, FC], f32, isOutput=False)
    w2 = nc.declare_dram_parameter("w2", [D_FF, D_MODEL], f32r, isOutput=False)
    b2t = nc.declare_dram_parameter("b2t", [128, MC], f32, isOutput=False)
    yt = nc.declare_dram_parameter("yt", [D_MODEL, C], f32, isOutput=True)

    xt_r = xt.rearrange("(dc p) n -> p dc n", p=128)
    w1_r = w1.rearrange("(dc p) f -> p dc f", p=128)
    w2_r = w2.rearrange("(fc p) m -> p fc m", p=128)
    yt_r = yt.rearrange("(mc p) n -> p mc n", p=128)

    Act = mybir.ActivationFunctionType

    with TileContext(nc) as tc:
        with (
            tc.tile_pool(name="consts", bufs=1) as consts,
            tc.tile_pool(name="xpool", bufs=2) as xpool,
            tc.tile_pool(name="w1pool", bufs=W1_BUFS) as w1pool,
            tc.tile_pool(name="w2pool", bufs=6) as w2pool,
            tc.tile_pool(name="htpool", bufs=1) as htpool,
            tc.tile_pool(name="ypool", bufs=2) as ypool,
            tc.tile_pool(name="psh", bufs=4, space="PSUM") as psh,
            tc.tile_pool(name="psy", bufs=3, space="PSUM") as psy,
        ):
            b1_sb = consts.tile([128, FC], f32)
            b2_sb = consts.tile([128, MC], f32)

            # software-pipelined loads: w1 pair DMAs issued 2 groups ahead of
            # use (crossing tile boundaries), xt issued one tile ahead, so
            # MM1 never starts a tile with cold buffers.
            from collections import deque

            NPAIR = FC // W1W
            w1_queue = deque()

            def issue_w1(fp, split=False):
                w1f = w1pool.tile([128, DC, 128 * W1W], f32r, tag="w1f")
                if split:
                    # four queues, first 128-col chunk first → the first
                    # matmul group can start as soon as ~0.5MB has landed
                    h = 32 * W1W
                    for i in range(4):
                        nc.sync.dma_start(
                            out=w1f[:, :, i * h : (i + 1) * h],
                            in_=w1_r[
                                :,
                                :,
                                fp * 128 * W1W + i * h : fp * 128 * W1W
                                + (i + 1) * h,
                            ],
                        )
                else:
                    nc.sync.dma_start(out=w1f, in_=w1_r[:, :, ts(fp, 128 * W1W)])
                w1_queue.append(w1f)

            xt_tiles = {}

            def issue_xt(t, split=False):
                tile = xpool.tile([128, DC, TILE_N], f32r, tag="xt_sb")
                nsplit = 4 if split else 2
                step = DC // nsplit
                for i in range(nsplit):
                    nc.sync.dma_start(
                        out=tile[:, i * step : (i + 1) * step, :],
                        in_=xt_r[:, i * step : (i + 1) * step, ts(t, TILE_N)],
                    )
                xt_tiles[t] = tile

            # w2 streaming plan: 3 quarters of the mp=0 group stay RESIDENT
            # across tiles (saves (nt-1)*3MB of HBM re-streaming); the rest
            # stream through a lookahead deque.
            N_RES = 3
            w2_queue = deque()
            w2_issued = [0]
            stream_plan = [
                (tt, mp, q)
                for tt in range(nt)
                for mp in range(MC // 2)
                for q in range(4)
                if not (mp == 0 and q < N_RES)
            ]
            # plan entries consumed strictly in order; index of first entry
            # belonging to each (t, mp) group:
            group_start = {}
            for k, (tt, mp, q) in enumerate(stream_plan):
                group_start.setdefault((tt, mp), k)

            def issue_w2():
                tt, mp, q = stream_plan[w2_issued[0]]
                w2q = w2pool.tile([128, 8, 256], f32r, tag="w2q")
                nc.sync.dma_start(
                    out=w2q, in_=w2_r[:, q * 8 : (q + 1) * 8, ts(mp, 256)]
                )
                w2_queue.append(w2q)
                w2_issued[0] += 1

            issue_xt(0, split=True)
            issue_w1(0, split=True)
            issue_w1(1)
            nc.scalar.dma_start(out=b1_sb, in_=b1t[:, :])
            nc.scalar.dma_start(out=b2_sb, in_=b2t[:, :])
            w2res = []

            def issue_w2res():
                q = len(w2res)
                w2rq = consts.tile([128, 8, 256], f32r, name=f"w2res{q}")
                nc.sync.dma_start(
                    out=w2rq, in_=w2_r[:, q * 8 : (q + 1) * 8, ts(0, 256)]
                )
                w2res.append(w2rq)

            issue_w2()
            issue_w2()

            for t in range(nt):
                xt_sb = xt_tiles.pop(t)

                ht = htpool.tile([128, FC, TILE_N], f32r, tag="ht")
                tile_q_base = t * (MC // 2) * 4
                for fp in range(NPAIR):
                    pos = t * NPAIR + fp + 2
                    if pos < nt * NPAIR:
                        issue_w1(pos % NPAIR)
                    if t == 0 and NPAIR - 4 <= fp < NPAIR - 4 + N_RES:
                        issue_w2res()
                    w1f = w1_queue.popleft()
                    for fi in range(W1W):
                        f = W1W * fp + fi
                        ph = psh.tile([128, TILE_N], f32, tag="ph")
                        for dc in range(DC):
                            nc.tensor.matmul(
                                ph,
                                w1f[:, dc, ts(fi, 128)],
                                xt_sb[:, dc, :],
                                start=(dc == 0),
                                stop=(dc == DC - 1),
                            )
                        nc.scalar.activation(
                            ht[:, f, :], ph, Act.Gelu, bias=b1_sb[:, f : f + 1]
                        )

                if t + 1 < nt:
                    issue_xt(t + 1)

                for mp in range(MC // 2):
                    nstream = 4 - N_RES if mp == 0 else 4
                    gs = group_start[(t, mp)]
                    # own quarters at group start; the 2-ahead issues move to
                    # mid-group (after mi=0) to smooth SP issue bursts
                    while w2_issued[0] < min(len(stream_plan), gs + nstream):
                        issue_w2()
                    if mp == 0:
                        quarters = list(w2res) + [
                            w2_queue.popleft() for _ in range(nstream)
                        ]
                    else:
                        quarters = [w2_queue.popleft() for _ in range(4)]
                    for mi in range(2):
                        m = 2 * mp + mi
                        py = psy.tile([128, TILE_N], f32, tag="py")
                        for fc in range(FC):
                            nc.tensor.matmul(
                                py,
                                quarters[fc // 8][:, fc % 8, ts(mi, 128)],
                                ht[:, fc, :],
                                start=(fc == 0),
                                stop=(fc == FC - 1),
                            )
                        y_sb = ypool.tile([128, TILE_N], f32, tag="y_sb")
                        nc.scalar.activation(
                            y_sb, py, Act.Identity, bias=b2_sb[:, m : m + 1]
                        )
                        nc.scalar.dma_start(
                            out=yt_r[:, m, ts(t, TILE_N)], in_=y_sb
                        )
                        if mi == 0:
                            while w2_issued[0] < min(
                                len(stream_plan), gs + nstream + 2
                            ):
                                issue_w2()

    _spread_waits(nc)
    return nc


def kernel(x, gate_w, W1, b1, W2, b2):
    global LAST_RESULT
    x = np.asarray(x, dtype=np.float32)
    gate_w = np.asarray(gate_w, dtype=np.float32)
    W1 = np.asarray(W1, dtype=np.float32)
    b1 = np.asarray(b1, dtype=np.float32)
    W2 = np.asarray(W2, dtype=np.float32)
    b2 = np.asarray(b2, dtype=np.float32)

    B, S, D = x.shape
    N = B * S
    E = gate_w.shape[0]
    xf = x.reshape(N, D)

    # ---- Gate: softmax + top-2 (host, fp32, matches jax semantics) ----
    logits = (xf @ gate_w.T) / np.float32(GATE_TEMP)
    m = logits.max(axis=1, keepdims=True)
    ex = np.exp(logits - m, dtype=np.float32)
    probs = ex / ex.sum(axis=1, keepdims=True, dtype=np.float32)
    order = np.argsort(-probs, axis=1, kind="stable")[:, :TOP_K]
    topk = np.take_along_axis(probs, order, axis=1)
    topk = topk / topk.sum(axis=1, keepdims=True, dtype=np.float32)

    # ---- Aux loss (faithful to reference) ----
    prob_mean = probs.mean(axis=0, dtype=np.float64)
    tokens_per_expert = (probs > 0).mean(axis=0, dtype=np.float64)
    aux_loss = np.float32(
        np.sum(prob_mean * tokens_per_expert) * E * LOAD_BALANCE_WEIGHT
    )

    # ---- Dispatch: token lists per expert ----
    tok_lists = []
    wgt_lists = []
    for e in range(E):
        sel_k0 = order[:, 0] == e
        sel_k1 = order[:, 1] == e
        toks = np.nonzero(sel_k0 | sel_k1)[0]
        w = np.where(sel_k0[toks], topk[toks, 0], topk[toks, 1])
        tok_lists.append(toks)
        wgt_lists.append(w.astype(np.float32))
    cmax = max(max(len(t) for t in tok_lists), 256)
    nt = (cmax + TILE_N_MAX - 1) // TILE_N_MAX
    tile_n = max(256, ((cmax + nt - 1) // nt + 7) // 8 * 8)
    C = nt * tile_n

    in_maps = []
    for e in range(E):
        toks = tok_lists[e]
        xt_e = np.zeros((D_MODEL, C), dtype=np.float32)
        xt_e[:, : len(toks)] = xf[toks].T
        in_maps.append(
            {
                "xt": np.ascontiguousarray(xt_e),
                "w1": np.ascontiguousarray(W1[e]),
                "b1t": np.ascontiguousarray(b1[e].reshape(FC, 128).T),
                "w2": np.ascontiguousarray(W2[e]),
                "b2t": np.ascontiguousarray(b2[e].reshape(MC, 128).T),
            }
        )

    nc = _build_nc(nt, tile_n)
    if TRACE:
        res = run_bass_kernel_spmd(
            nc, in_maps, core_ids=list(range(N_EXPERTS)), trace=True
        )
    else:
        res = run_bass_kernel_spmd(nc, in_maps, core_ids=list(range(N_EXPERTS)))
    LAST_RESULT = res

    out = np.zeros((N, D_MODEL), dtype=np.float32)
    for e in range(E):
        toks = tok_lists[e]
        if len(toks) == 0:
            continue
        y_e = res.results[e]["yt"][:, : len(toks)].T
        out[toks] += wgt_lists[e][:, None] * y_e
    return out.reshape(B, S, D_MODEL), aux_loss
